# revision 55
# baseline (speedup 1.0000x reference)
"""MoE head (router top-2 + per-expert GELU FFN) on 8 TRN2 NeuronCores.

Strategy (sharding hint: expert parallel with top-k dispatch):
  - Host computes the (tiny) router in float64: logits = x @ Wr.T,
    top-2 experts per token, softmax-over-top2 gate weights.
  - Tokens are gathered per expert on the host ("all-to-all dispatch"),
    padded to a fixed capacity C, and each of the 8 cores runs ONE
    expert's FFN over its gathered tokens:
        y_tok = gate * (gelu(x_tok @ W_in[e].T) @ W_out[e])
  - Host scatter-adds the two expert contributions per token.

  Device kernel (SPMD, one program, per-core data):
    mm1: h^T[dhid, tok] = W_in^T-chunks.T @ x^T-chunks  (PE, accumulate K=512)
    gelu: ACT engine, PSUM -> SBUF (fp16)
    mm2: y^T[dh, tok] = W_out-chunks @ h^T-chunks       (PE, accumulate K=2048)
    gate: DVE elementwise multiply vs pre-broadcast G[p,tok], then DMA out.

  Matmuls run in float16 (host-cast inputs). Measured on HW (sustained,
  loop-slope): fp16/bf16 ~253 ns vs f32r ~294 ns per [128x128]@[128x512]
  matmul chain (fp16 hits 1 row/cycle @2.4GHz; f32r pays ~19% more per
  row), and fp16 inputs halve the HBM stream (10.4 -> 5.2 MB/core, fully
  hidden under compute vs ~12 us exposed for f32r). fp16 also halves the
  y drain (y is written fp16, upcast on host). mm2 is computed
  TRANSPOSED (y^T[dh, tok], moving dim = ragged token group) so both
  matmuls are MAC-optimal: rows = 64*cap each, no ceil(cap/128) subtile
  waste. The per-token gate is applied to y^T as an elementwise DVE
  multiply against a host-pre-broadcast G[p, tok] = gate[tok] tile
  (280KB fp16, free under the weight stream). End-to-end error vs the
  fp32 reference: ~5.5e-4 scale-relative absmax (tolerance 2e-2).

  Timing on HW (test.py loop-slope, quiet machine): ~62-67 us/kernel vs
  98.5-101.5 us for the f32r baseline (~1.5x). The estimator is noisy
  and upward-skewed (axon RPC jitter + shared-terminal contention:
  identical code sampled 61-88 us across windows), so test.py reports
  the median of 5 independent median-of-30 slope rounds. CoreSim
  (no_exec) budget at cap=1092: PE busy 58.45 us (MAC floor 58.24, zero
  mid-stream gaps), DMA-wait head ~3.3 us (2 serial dma issue+DGE
  chains; w_in lands before xg since Ldweights precedes the matmul),
  drain tail ~2.6 us (last-chunk PSUM accumulation split in half so
  half the DVE+DMA drain overlaps the final matmuls; groups sorted
  descending so the last drain is the smallest group). PSUM ps1=5/ps2=3
  and the staggered phase order (mm1 g0, mm1 g1, mm2 g0, ...) scanned
  optimal in sim. Rejected by measurement: finer-grained head DMA and
  off-SP DMA issue (each dma_start costs ~565 ns of sequencer issue
  time; more/finer descriptors push the whole stream later; <512B
  contiguous chunks transfer at half rate), on-device gate broadcast
  via K=1 matmul (schedule poison; host pre-broadcast is free),
  expert-pair token balancing (needs 2 weight sets/core -> 9.2MB
  stream re-exposes DMA), fp8 DoubleRow (2x PE rate but ~3-10% error
  >> 2e-2 tolerance), PE prewarm (sim models ramp from t=0).

  MOE_SPLIT=1 (the default) selects the 2-slot dhid-split variant:
  each core computes half the hidden dim (1024) of one big + one small
  expert, so slot caps are (max big, max small) = (1092, 1004) instead
  of every core paying the global max -- 5760 fewer PE rows (2.4us)
  with no weight duplication (two half-experts = one full expert's
  bytes); the host sums the two half-dhid partial y's per expert (gate
  is linear, applied in each half). Needs coarse multi-kc rearrange
  DMA descriptors: a first fine-grained version lost 7.9us to SP
  sequencer saturation (~48us of issue time at ~565ns per dma_start).
  Sim: 64.13 vs 64.29us for single-expert/core; HW: equal-or-better in
  every paired window (the +1.5MB/core stream stays hidden, incl. 8-core
  self-contention). rel err 5.31e-4. MOE_SPLIT=0 falls back to
  1-expert/core. Caveat: >2 loop-program executables in one process can
  crash the axon runtime -- bench configs in separate processes.
"""

import os

import numpy as np

P = 128
DH = 512
DHID = 2048
NE = 8
TOPK = 2
KC1 = DH // P  # k-chunks for mm1
KC2 = DHID // P  # k-chunks for mm2
N_CORES = 8

DEFAULT_CAP = 1152  # tokens per expert; E[count] = T*K/NE = 1024 for T=4096

_prog_cache: dict[tuple, object] = {}
LAST_EXEC_NS = None  # filled when MOE_TRACE=1
LAST_RESULTS = None
LAST_CAP = None


def _dtype_mode() -> str:
    return os.environ.get("MOE_DTYPE", "fp16")  # fp16 | f32r | bf16 | fp32


def _groups_of(c: int) -> list[int]:
    """Token groups: each in [256, 512] so the f32r matmul moving dim stays
    >= 256 (4x slower below). All groups except the last are multiples of
    128, so every group offset is 128-aligned; the last group absorbs any
    remainder (its final mm2 subtile is partial-M)."""
    rem = c % 128
    if rem:
        last = 256 + rem  # in (256, 384)
        body = c - last
    else:
        last = None
        body = c
    gs = []
    left = body
    while left > 640:
        gs.append(512)
        left -= 512
    if left > 512:  # 640+128k leftovers: split into two >=256 chunks
        hi = (left // 256) * 128
        gs.extend([left - hi, hi])
    elif left:
        gs.append(left)
    if last is not None:
        gs.append(last)
    if os.environ.get("MOE_G0FIRST", "0") == "1" and len(gs) > 2:
        # Smallest group first: the PE's first mm1 phase needs the fewest
        # input bytes, so it starts sooner after the DMA head.
        i = gs.index(min(gs[:-1]))
        gs.insert(0, gs.pop(i))
    if os.environ.get("MOE_GSORT", "1") == "1":
        # Largest first, smallest last: the final mm2 drain (DVE + y DMA
        # serial after the last matmul) scales with the last group's size.
        gs.sort(reverse=True)
    return gs


def _build_program(
    cap: int, mode: str, reps: int = 1, loop_n: int = 0, loop_scope: str = "all"
):
    """loop_n > 0 wraps the body in a runtime For_i loop (timing only).
    loop_scope="compute" hoists the DMA+round stage out of the loop."""
    import contextlib

    import concourse.mybir as mybir
    import concourse.tile as tile
    from concourse import bacc

    f32 = mybir.dt.float32
    mm_dt = {
        "f32r": mybir.dt.float32r,
        "bf16": mybir.dt.bfloat16,
        "fp16": mybir.dt.float16,
        "fp32": mybir.dt.float32,
    }[mode]

    nc = bacc.Bacc(None, target_bir_lowering=False, debug=False)
    # f32r inputs are declared as float32r (same 4-byte layout as fp32; numpy
    # side is np.float32). The PE truncates fp32 -> f32r internally, so raw
    # fp32 bytes DMA'd straight in give the same result as an explicit
    # rounding pass -- verified on HW. fp16 inputs are cast on the HOST
    # (np.float16) and DMA'd directly: half the HBM bytes, no on-device cast.
    if mode == "f32r":
        in_dt = mybir.dt.float32r
    elif mode == "fp16":
        in_dt = mybir.dt.float16
    else:
        in_dt = f32
    xg = nc.declare_dram_parameter("xg", [DH, cap], in_dt, isOutput=False)
    w_in_t = nc.declare_dram_parameter("w_in_t", [DH, DHID], in_dt, isOutput=False)
    w_out = nc.declare_dram_parameter("w_out", [DHID, DH], in_dt, isOutput=False)
    gcols = -(-cap // P)
    # mm2t: y is computed transposed (y^T[dh, tok]) so mm2's moving dim is
    # the ragged token group instead of DH -- no partial-subtile waste
    # (mm2 rows drop from ceil(cap/128)*128*16*4 to cap*64, MAC-optimal).
    # The per-token gate becomes a per-COLUMN scale, applied via a
    # PE-built broadcast tile G[p, tok] = gate[tok].
    mm2t = mode == "fp16" and os.environ.get("MOE_MM2T", "1") == "1"
    gate_dt = mybir.dt.float16 if mm2t else f32
    # mm2t: gate arrives pre-broadcast from the host as G[p, tok] = gate[tok]
    # (280KB fp16 -- negligible next to the 4MB weight stream) so no
    # on-device broadcast build is needed.
    gate_shape = [P, cap] if mm2t else [gcols * P]
    gate = nc.declare_dram_parameter("gate", gate_shape, gate_dt, isOutput=False)
    # fp16 y halves the output drain; the host upcasts + scatter-adds in f32.
    out_dt = mybir.dt.float16 if mode == "fp16" else f32
    y_shape = [DH, cap] if mm2t else [cap, DH]
    y = nc.declare_dram_parameter("y", y_shape, out_dt, isOutput=True)

    groups = _groups_of(cap)
    gelu = mybir.ActivationFunctionType.Gelu

    with tile.TileContext(nc) as tc:
        with (
            tc.tile_pool(name="persist", bufs=1) as persist,
            tc.tile_pool(name="stage", bufs=3) as stage,
            # hbuf=1 for very large caps (all-tokens-to-one-expert fallback)
            # so the SBUF budget still closes; slower but correct.
            tc.tile_pool(
                name="hbuf",
                bufs=int(os.environ.get("MOE_HBUF", "2" if cap <= 2304 else "1")),
            ) as hbuf,
            tc.tile_pool(name="ybuf", bufs=int(os.environ.get("MOE_YBUF", "4"))) as ybuf,
            tc.tile_pool(name="ps1", bufs=int(os.environ.get("MOE_PS1", "5")), space="PSUM") as ps1,
            tc.tile_pool(name="ps2", bufs=int(os.environ.get("MOE_PS2", "3")), space="PSUM") as ps2,
        ):
            state = {}

            def emit_loads():
                # ---- direct DMA into matmul-dtype tiles (no staging) ----
                if mode == "bf16":
                    # bf16 needs an on-device cast: stage fp32 then DVE copy.
                    xg_r = persist.tile([P, KC1, cap], mm_dt, tag="xg_r")
                    for kc in range(KC1):
                        stg = stage.tile([P, cap], f32, tag="stg_x")
                        nc.sync.dma_start(out=stg, in_=xg[kc * P : (kc + 1) * P, :])
                        nc.vector.tensor_copy(xg_r[:, kc, :], stg)
                    w_in_r = persist.tile([P, KC1, DHID], mm_dt, tag="w_in_r")
                    for kc in range(KC1):
                        stg = stage.tile([P, DHID], f32, tag="stg_wi")
                        nc.sync.dma_start(
                            out=stg, in_=w_in_t[kc * P : (kc + 1) * P, :]
                        )
                        nc.vector.tensor_copy(w_in_r[:, kc, :], stg)
                    w_out_r = persist.tile([P, KC2, DH], mm_dt, tag="w_out_r")
                    for kc in range(KC2):
                        stg = stage.tile([P, DH], f32, tag="stg_wo")
                        nc.sync.dma_start(
                            out=stg, in_=w_out[kc * P : (kc + 1) * P, :]
                        )
                        nc.vector.tensor_copy(w_out_r[:, kc, :], stg)
                elif os.environ.get("MOE_FINEDMA", "1") == "1":
                    # First-use-ordered fine-grained input stream: the PE's
                    # first work (mm1 group0, d0-3) needs only xg[:,g0] and
                    # w_in quarter q0 -- land those ~2MB first so the PE
                    # starts ~6us in instead of waiting on whole chunks.
                    xg_r = persist.tile([P, KC1, cap], in_dt, tag="xg_r")
                    w_in_r = persist.tile([P, KC1, DHID], in_dt, tag="w_in_r")
                    w_out_r = persist.tile([P, KC2, DH], in_dt, tag="w_out_r")
                    Q = DHID // 4
                    g_offs = []
                    o = 0
                    for gsz in groups:
                        g_offs.append((o, gsz))
                        o += gsz

                    def dma_xg(kc, gi, eng=None):
                        o, gsz = g_offs[gi]
                        (eng or nc.sync).dma_start(
                            out=xg_r[:, kc, o : o + gsz],
                            in_=xg[kc * P : (kc + 1) * P, o : o + gsz],
                        )

                    # Issue weight streams from otherwise-idle engine
                    # sequencers: each dma_start costs ~565-667ns of issue
                    # time on its engine, and SP serializes the whole input
                    # stream otherwise. ACT is idle until the first gelu
                    # (~4us in), DVE until the first mm2 (~20us in).
                    # MOE_ENG: 0 = all DMA issue on SP; 1 = w_in on ACT +
                    # w_out on Pool; 2 = only the head w_in chunks on ACT
                    # (ACT is idle before the first gelu; later issues would
                    # block the gelu stream), rest on SP.
                    eng_mode = os.environ.get("MOE_ENG", "0")
                    win_eng = nc.scalar if eng_mode == "1" else nc.sync
                    wout_eng = nc.gpsimd if eng_mode == "1" else nc.sync
                    head_eng = nc.scalar if eng_mode in ("1", "2") else nc.sync

                    def dma_win_cols(kc, c0, c1, eng=None):
                        (eng or win_eng).dma_start(
                            out=w_in_r[:, kc, c0:c1],
                            in_=w_in_t[kc * P : (kc + 1) * P, c0:c1],
                        )

                    def dma_win(kc, q):
                        dma_win_cols(kc, q * Q, (q + 1) * Q)

                    if os.environ.get("MOE_FINEHEAD", "0") == "1":
                        # Minimal first-matmul set: xg group0 + w_in d0 cols
                        # only (32KB/chunk fp16), so the PE starts earliest.
                        for kc in range(KC1):
                            dma_xg(kc, 0)
                            dma_win_cols(kc, 0, P)
                        for kc in range(KC1):
                            dma_win_cols(kc, P, Q)
                            dma_xg(kc, 1)
                    elif os.environ.get("MOE_HEADI", "1") == "1":
                        winfirst = os.environ.get("MOE_WINFIRST", "1") == "1"
                        # MOE_ENG=3: xg group0 issues on ACT (idle until the
                        # first gelu) concurrently with w_in q0 on SP, so the
                        # two first-matmul inputs don't serialize on one
                        # sequencer.
                        xg0_eng = nc.scalar if eng_mode == "3" else None
                        for kc in range(KC1):
                            # winfirst: the PE's Ldweights needs w_in before
                            # the matmul touches xg -- land it first.
                            if winfirst:
                                dma_win_cols(kc, 0, Q, eng=head_eng)
                                dma_xg(kc, 0, eng=xg0_eng)
                            else:
                                dma_xg(kc, 0, eng=xg0_eng)
                                dma_win_cols(kc, 0, Q, eng=head_eng)
                        for kc in range(KC1):
                            dma_win(kc, 1)
                            dma_xg(kc, 1)
                    else:
                        for kc in range(KC1):
                            dma_xg(kc, 0)
                        for kc in range(KC1):
                            dma_win(kc, 0)
                        for kc in range(KC1):
                            dma_win(kc, 1)
                        for kc in range(KC1):
                            dma_xg(kc, 1)
                    if os.environ.get("MOE_FINEHEAD", "0") == "1":
                        for kc in range(KC1):
                            dma_win(kc, 1)
                    for kc in range(KC1):
                        dma_win(kc, 2)
                    for kc in range(KC1):
                        dma_win(kc, 3)
                    for kc2 in range(KC2 // 2):
                        wout_eng.dma_start(
                            out=w_out_r[:, kc2 * 2 : (kc2 + 1) * 2, :],
                            in_=w_out.rearrange("(kc p) d -> p kc d", p=P)[
                                :, kc2 * 2 : (kc2 + 1) * 2, :
                            ],
                        )
                    for gi in range(2, len(groups)):
                        for kc in range(KC1):
                            dma_xg(kc, gi)
                else:
                    xg_r = persist.tile([P, KC1, cap], in_dt, tag="xg_r")
                    for kc in range(KC1):
                        nc.sync.dma_start(
                            out=xg_r[:, kc, :], in_=xg[kc * P : (kc + 1) * P, :]
                        )
                    w_in_r = persist.tile([P, KC1, DHID], in_dt, tag="w_in_r")
                    for kc in range(KC1):
                        for h2 in range(2):  # halves for earlier first-use
                            nc.sync.dma_start(
                                out=w_in_r[:, kc, h2 * (DHID // 2) : (h2 + 1) * (DHID // 2)],
                                in_=w_in_t[
                                    kc * P : (kc + 1) * P,
                                    h2 * (DHID // 2) : (h2 + 1) * (DHID // 2),
                                ],
                            )
                    w_out_r = persist.tile([P, KC2, DH], in_dt, tag="w_out_r")
                    for kc4 in range(KC2 // 4):
                        nc.sync.dma_start(
                            out=w_out_r[:, kc4 * 4 : (kc4 + 1) * 4, :],
                            in_=w_out.rearrange("(kc p) d -> p kc d", p=P)[
                                :, kc4 * 4 : (kc4 + 1) * 4, :
                            ],
                        )
                if mm2t:
                    gate_sb = persist.tile([P, cap], gate_dt, tag="gate_sb")
                    nc.sync.dma_start(out=gate_sb, in_=gate[:, :])
                else:
                    gate_sb = persist.tile([P, gcols], f32, tag="gate_sb")
                    nc.sync.dma_start(
                        out=gate_sb, in_=gate.rearrange("(g p) -> p g", p=P)
                    )
                state.update(
                    xg_r=xg_r, w_in_r=w_in_r, w_out_r=w_out_r, gate_sb=gate_sb
                )

            def emit_compute():
                # ---- mm1 -> gelu -> mm2 -> gate -> out, per token-group ----
                # Phase order is staggered (mm1 g0, mm1 g1, mm2 g0, mm1 g2,
                # mm2 g1, mm2 g2) so the first mm2 starts ~2 mm1-phases into
                # the kernel, giving the 4MB w_out DMA stream time to land
                # without stalling the PE. Needs 2 live h tiles (hbuf=2).
                xg_r, w_in_r = state["xg_r"], state["w_in_r"]
                w_out_r, gate_sb = state["w_out_r"], state["gate_sb"]
                offs = []
                off = 0
                for gsz in groups:
                    offs.append(off)
                    off += gsz
                h_tiles = {}

                g_bc = gate_sb if mm2t else None

                def mm1_phase(gi):
                    gsz, off = groups[gi], offs[gi]
                    h_r = hbuf.tile([P, KC2, gsz], mm_dt, tag="h_r")
                    h_tiles[gi] = h_r
                    for d in range(KC2):
                        ps = ps1.tile([P, gsz], f32, tag="p1")
                        for kc in range(KC1):
                            nc.tensor.matmul(
                                ps,
                                w_in_r[:, kc, d * P : (d + 1) * P],
                                xg_r[:, kc, off : off + gsz],
                                start=(kc == 0),
                                stop=(kc == KC1 - 1),
                            )
                        nc.scalar.activation(h_r[:, d, :], ps, gelu)

                def mm2_phase(gi, tail=False):
                    gsz, off = groups[gi], offs[gi]
                    h_r = h_tiles.pop(gi)
                    if mm2t:
                        # y^T[dh_c, tok] = sum_d W_out[d, dh_c]^T @ h^T[d, tok]
                        # moving dim = tokens (ragged-exact), then gate via
                        # elementwise multiply with the G broadcast tile.
                        tsplit = (
                            tail
                            and gsz > 128
                            and os.environ.get("MOE_TAILSPLIT", "1") == "1"
                        )
                        for c in range(KC1):
                            if tsplit and c == KC1 - 1:
                                # Split the very last chunk's accumulation so
                                # half its DVE+DMA drain overlaps the PE's
                                # final matmuls instead of trailing them.
                                h1 = gsz // 2 // 64 * 64
                                parts = [(0, h1), (h1, gsz - h1)]
                            else:
                                parts = [(0, gsz)]
                            for po, pl in parts:
                                pt = ps2.tile([P, pl], f32, tag="p2")
                                for d in range(KC2):
                                    nc.tensor.matmul(
                                        pt,
                                        w_out_r[:, d, c * P : (c + 1) * P],
                                        h_r[:, d, po : po + pl],
                                        start=(d == 0),
                                        stop=(d == KC2 - 1),
                                    )
                                y_sb = ybuf.tile([P, pl], out_dt, tag="y_sb")
                                nc.vector.scalar_tensor_tensor(
                                    y_sb,
                                    pt,
                                    1.0,
                                    g_bc[:, off + po : off + po + pl],
                                    mybir.AluOpType.mult,
                                    mybir.AluOpType.mult,
                                )
                                nc.sync.dma_start(
                                    out=y[
                                        c * P : (c + 1) * P,
                                        off + po : off + po + pl,
                                    ],
                                    in_=y_sb,
                                )
                        return
                    for s in range(-(-gsz // P)):
                        m = min(P, gsz - s * P)  # last subtile may be partial
                        pt = ps2.tile([P, DH], f32, tag="p2")
                        for d in range(KC2):
                            nc.tensor.matmul(
                                pt[:m, :],
                                h_r[:, d, s * P : s * P + m],
                                w_out_r[:, d, :],
                                start=(d == 0),
                                stop=(d == KC2 - 1),
                            )
                        tok0 = off + s * P
                        y_sb = ybuf.tile([P, DH], out_dt, tag="y_sb")
                        nc.vector.tensor_scalar_mul(
                            y_sb[:m, :], pt[:m, :], gate_sb[:m, tok0 // P : tok0 // P + 1]
                        )
                        nc.sync.dma_start(out=y[tok0 : tok0 + m, :], in_=y_sb[:m, :])

                n_g = len(groups)
                if n_g == 1 or os.environ.get("MOE_STAGGER", "1") != "1":
                    for gi in range(n_g):
                        mm1_phase(gi)
                        mm2_phase(gi, tail=(gi == n_g - 1))
                else:
                    mm1_phase(0)
                    mm1_phase(1)
                    for gi in range(2, n_g):
                        mm2_phase(gi - 2)
                        mm1_phase(gi)
                    mm2_phase(n_g - 2)
                    mm2_phase(n_g - 1, tail=True)

            if loop_n and loop_scope == "loads":
                # DMA-stream-only loop: measures the pure input stream time.
                with tc.For_i(0, loop_n, 1):
                    for _rep in range(reps):
                        emit_loads()
                emit_compute()
            elif loop_n and loop_scope == "compute":
                emit_loads()
                with tc.For_i(0, loop_n, 1):
                    for _rep in range(reps):
                        emit_compute()
            elif loop_n:
                with tc.For_i(0, loop_n, 1):
                    for _rep in range(reps):
                        emit_loads()
                        emit_compute()
            else:
                for _rep in range(reps):
                    emit_loads()
                    emit_compute()

    nc.compile()
    return nc


def _build_program_split(
    caps, mode: str = "fp16", loop_n: int = 0, loop_scope: str = "all"
):
    """2-slot dhid-split: each core runs HALF the hidden dim (1024) of two
    different experts -- slot0 one of the 4 biggest, slot1 one of the 4
    smallest -- so slot capacities are (max big, max small) instead of every
    core paying the global max. Same total weight bytes as 1-expert/core;
    only xg/gate/y are duplicated (~+1.5MB/core). Host sums the two half-dhid
    partial y's per expert (gate is linear, applied in each half)."""
    import concourse.mybir as mybir
    import concourse.tile as tile
    from concourse import bacc

    assert mode == "fp16"
    f32 = mybir.dt.float32
    fp16 = mybir.dt.float16
    DHH = DHID // 2
    KCH = DHH // P  # 8 k-chunks for mm2 per segment
    S = list(caps)

    nc = bacc.Bacc(None, target_bir_lowering=False, debug=False)
    xg = [
        nc.declare_dram_parameter(f"xg{s}", [DH, S[s]], fp16, isOutput=False)
        for s in range(2)
    ]
    wi = [
        nc.declare_dram_parameter(f"wi{s}", [DH, DHH], fp16, isOutput=False)
        for s in range(2)
    ]
    wo = [
        nc.declare_dram_parameter(f"wo{s}", [DHH, DH], fp16, isOutput=False)
        for s in range(2)
    ]
    gt = [
        nc.declare_dram_parameter(f"g{s}", [P, S[s]], fp16, isOutput=False)
        for s in range(2)
    ]
    y = [
        nc.declare_dram_parameter(f"y{s}", [DH, S[s]], fp16, isOutput=True)
        for s in range(2)
    ]
    groups = [_groups_of(S[s]) for s in range(2)]
    gelu = mybir.ActivationFunctionType.Gelu

    with tile.TileContext(nc) as tc:
        with (
            tc.tile_pool(name="persist", bufs=1) as persist,
            tc.tile_pool(
                name="hbuf", bufs=int(os.environ.get("MOE_HBUF", "2"))
            ) as hbuf,
            tc.tile_pool(
                name="ybuf", bufs=int(os.environ.get("MOE_YBUF", "4"))
            ) as ybuf,
            tc.tile_pool(
                name="ps1", bufs=int(os.environ.get("MOE_PS1", "5")), space="PSUM"
            ) as ps1,
            tc.tile_pool(
                name="ps2", bufs=int(os.environ.get("MOE_PS2", "3")), space="PSUM"
            ) as ps2,
        ):
            state = {}

            def emit_loads():
                QW = DHH // 4
                xg_r, wi_r, wo_r, g_r = [], [], [], []
                for s in range(2):
                    xg_r.append(
                        persist.tile(
                            [P, KC1, S[s]], fp16, tag=f"xg_r{s}", name=f"xg_r{s}"
                        )
                    )
                    wi_r.append(
                        persist.tile(
                            [P, KC1, DHH], fp16, tag=f"wi_r{s}", name=f"wi_r{s}"
                        )
                    )
                    wo_r.append(
                        persist.tile(
                            [P, KCH, DH], fp16, tag=f"wo_r{s}", name=f"wo_r{s}"
                        )
                    )
                    g_r.append(
                        persist.tile([P, S[s]], fp16, tag=f"g_r{s}", name=f"g_r{s}")
                    )
                offs = []
                for s in range(2):
                    o, oo = 0, []
                    for gsz in groups[s]:
                        oo.append(o)
                        o += gsz
                    offs.append(oo)

                # Coarse multi-kc descriptors (one dma_start per logical
                # block) everywhere except the fine seg0 head: each
                # dma_start costs ~565ns of SP issue time and the split
                # program has ~2x the blocks of the single-expert one.
                def dma_xg(s, gi):
                    o, gsz = offs[s][gi], groups[s][gi]
                    nc.sync.dma_start(
                        out=xg_r[s][:, :, o : o + gsz],
                        in_=xg[s].rearrange("(kc p) c -> p kc c", p=P)[
                            :, :, o : o + gsz
                        ],
                    )

                def dma_wi(s, q):
                    nc.sync.dma_start(
                        out=wi_r[s][:, :, q * QW : (q + 1) * QW],
                        in_=wi[s].rearrange("(kc p) c -> p kc c", p=P)[
                            :, :, q * QW : (q + 1) * QW
                        ],
                    )

                def dma_wo(s, half):
                    k2 = KCH // 2
                    nc.sync.dma_start(
                        out=wo_r[s][:, half * k2 : (half + 1) * k2, :],
                        in_=wo[s].rearrange("(kc p) d -> p kc d", p=P)[
                            :, half * k2 : (half + 1) * k2, :
                        ],
                    )

                # seg0 head: fine per-kc chunks, w_in before xg (winfirst).
                # MOE_HEADQ2=1 widens the head w_in chunks to q0+q1 (4
                # d-chunks of PE runway instead of 2) to cover the coarse
                # stream's arrival.
                hq = 2 * QW if os.environ.get("MOE_HEADQ2", "1") == "1" else QW
                for kc in range(KC1):
                    nc.sync.dma_start(
                        out=wi_r[0][:, kc, 0:hq], in_=wi[0][kc * P : (kc + 1) * P, 0:hq]
                    )
                    nc.sync.dma_start(
                        out=xg_r[0][:, kc, 0 : groups[0][0]],
                        in_=xg[0][kc * P : (kc + 1) * P, 0 : groups[0][0]],
                    )
                # All remaining w_in quarters straight after the head: the
                # 8-d mm1 burns a 2-d quarter every ~1.7us, while xg group1
                # isn't touched until ~9us in.
                if hq == QW:
                    dma_wi(0, 1)
                dma_wi(0, 2)
                dma_wi(0, 3)
                dma_xg(0, 1)
                dma_wo(0, 0)
                dma_wo(0, 1)
                for gi in range(2, len(groups[0])):
                    dma_xg(0, gi)
                nc.sync.dma_start(out=g_r[0], in_=gt[0][:, :])
                dma_wi(1, 0)
                dma_xg(1, 0)
                dma_wi(1, 1)
                dma_wi(1, 2)
                dma_wi(1, 3)
                dma_xg(1, 1)
                dma_wo(1, 0)
                dma_wo(1, 1)
                for gi in range(2, len(groups[1])):
                    dma_xg(1, gi)
                nc.sync.dma_start(out=g_r[1], in_=gt[1][:, :])
                state.update(xg_r=xg_r, wi_r=wi_r, wo_r=wo_r, g_r=g_r, offs=offs)

            def emit_compute():
                xg_r, wi_r = state["xg_r"], state["wi_r"]
                wo_r, g_r, offs = state["wo_r"], state["g_r"], state["offs"]
                h_tiles = {}

                def mm1_phase(s, gi):
                    gsz, off = groups[s][gi], offs[s][gi]
                    h_r = hbuf.tile([P, KCH, gsz], fp16, tag="h_r")
                    h_tiles[(s, gi)] = h_r
                    for d in range(KCH):
                        ps = ps1.tile([P, gsz], f32, tag="p1")
                        for kc in range(KC1):
                            nc.tensor.matmul(
                                ps,
                                wi_r[s][:, kc, d * P : (d + 1) * P],
                                xg_r[s][:, kc, off : off + gsz],
                                start=(kc == 0),
                                stop=(kc == KC1 - 1),
                            )
                        nc.scalar.activation(h_r[:, d, :], ps, gelu)

                def mm2_phase(s, gi, tail=False):
                    gsz, off = groups[s][gi], offs[s][gi]
                    h_r = h_tiles.pop((s, gi))
                    for c in range(KC1):
                        if tail and c == KC1 - 1 and gsz > 128:
                            h1 = gsz // 2 // 64 * 64
                            parts = [(0, h1), (h1, gsz - h1)]
                        else:
                            parts = [(0, gsz)]
                        for po, pl in parts:
                            pt = ps2.tile([P, pl], f32, tag="p2")
                            for d in range(KCH):
                                nc.tensor.matmul(
                                    pt,
                                    wo_r[s][:, d, c * P : (c + 1) * P],
                                    h_r[:, d, po : po + pl],
                                    start=(d == 0),
                                    stop=(d == KCH - 1),
                                )
                            y_sb = ybuf.tile([P, pl], fp16, tag="y_sb")
                            nc.vector.scalar_tensor_tensor(
                                y_sb,
                                pt,
                                1.0,
                                g_r[s][:, off + po : off + po + pl],
                                mybir.AluOpType.mult,
                                mybir.AluOpType.mult,
                            )
                            nc.sync.dma_start(
                                out=y[s][
                                    c * P : (c + 1) * P, off + po : off + po + pl
                                ],
                                in_=y_sb,
                            )

                phases = [(0, gi) for gi in range(len(groups[0]))] + [
                    (1, gi) for gi in range(len(groups[1]))
                ]
                n_p = len(phases)
                mm1_phase(*phases[0])
                mm1_phase(*phases[1])
                for i in range(2, n_p):
                    mm2_phase(*phases[i - 2])
                    mm1_phase(*phases[i])
                mm2_phase(*phases[n_p - 2])
                mm2_phase(*phases[n_p - 1], tail=True)

            if loop_n and loop_scope == "loads":
                with tc.For_i(0, loop_n, 1):
                    emit_loads()
                emit_compute()
            elif loop_n and loop_scope == "compute":
                emit_loads()
                with tc.For_i(0, loop_n, 1):
                    emit_compute()
            elif loop_n:
                with tc.For_i(0, loop_n, 1):
                    emit_loads()
                    emit_compute()
            else:
                emit_loads()
                emit_compute()

    nc.compile()
    return nc


def _get_program(cap, mode: str):
    key = (tuple(cap) if isinstance(cap, (tuple, list)) else cap, mode)
    if key not in _prog_cache:
        if isinstance(cap, (tuple, list)):
            _prog_cache[key] = _build_program_split(cap, mode)
        else:
            _prog_cache[key] = _build_program(cap, mode)
    return _prog_cache[key]


def _kernel_split(x, tok_lists, gate_lists, W_in, W_out, mode):
    """Dispatch for the 2-slot dhid-split program: core k runs half k%2 of
    big-expert ord[k//2] (slot0) and of small-expert ord[4+k//2] (slot1)."""
    global LAST_EXEC_NS, LAST_RESULTS, LAST_CAP
    from concourse.bass_utils import run_bass_kernel_spmd

    DHH = DHID // 2
    ordr = sorted(range(NE), key=lambda e: -len(tok_lists[e]))
    S1 = max(-(-len(tok_lists[ordr[i]]) // 4) * 4 for i in range(4))
    S2 = max(-(-len(tok_lists[ordr[i]]) // 4) * 4 for i in range(4, 8))
    S1, S2 = max(S1, 384), max(S2, 384)
    LAST_CAP = (S1, S2)
    nc = _get_program((S1, S2), mode)

    def seg_inputs(e, half, cap):
        toks = tok_lists[e]
        hs = slice(half * DHH, (half + 1) * DHH)
        xg = np.zeros((DH, cap), np.float16)
        xg[:, : len(toks)] = x[toks].T.astype(np.float16)
        g = np.zeros((cap,), np.float16)
        g[: len(toks)] = gate_lists[e].astype(np.float16)
        g = np.ascontiguousarray(np.broadcast_to(g, (P, cap)))
        return {
            "xg": xg,
            "wi": np.ascontiguousarray(W_in[e][hs].T.astype(np.float16)),
            "wo": np.ascontiguousarray(W_out[e][hs].astype(np.float16)),
            "g": g,
        }

    in_maps = []
    for k in range(N_CORES):
        m = {}
        for s, (e, cap) in enumerate(
            ((ordr[k // 2], S1), (ordr[4 + k // 2], S2))
        ):
            seg = seg_inputs(e, k % 2, cap)
            m.update({f"{name}{s}": v for name, v in seg.items()})
        in_maps.append(m)

    trace = os.environ.get("MOE_TRACE", "0") == "1"
    res = run_bass_kernel_spmd(
        nc,
        in_maps,
        list(range(N_CORES)),
        trace=trace,
        trace_cores=list(range(N_CORES)) if trace else None,
    )
    LAST_EXEC_NS = res.exec_time_ns
    LAST_RESULTS = res

    T = x.shape[0]
    out = np.zeros((T, DH), np.float32)
    for k in range(N_CORES):
        for s, e in enumerate((ordr[k // 2], ordr[4 + k // 2])):
            toks = tok_lists[e]
            if len(toks):
                out[toks] += res.results[k][f"y{s}"][:, : len(toks)].T.astype(
                    np.float32
                )
    return out


def kernel(x, Wr, W_in, W_out):
    global LAST_EXEC_NS, LAST_RESULTS
    from concourse.bass_utils import run_bass_kernel_spmd

    x = np.ascontiguousarray(np.asarray(x), dtype=np.float32)
    Wr = np.asarray(Wr, dtype=np.float32)
    W_in = np.asarray(W_in, dtype=np.float32)
    W_out = np.asarray(W_out, dtype=np.float32)
    T = x.shape[0]

    # ---- host router (fp64: strictly more accurate than the fp32 ref) ----
    logits = x.astype(np.float64) @ Wr.astype(np.float64).T  # (T, NE)
    part = np.argpartition(-logits, TOPK - 1, axis=1)[:, :TOPK]
    vals = np.take_along_axis(logits, part, axis=1)
    order = np.argsort(-vals, axis=1, kind="stable")
    idx = np.take_along_axis(part, order, axis=1)  # (T, 2) desc
    ar = np.arange(T)
    v1 = logits[ar, idx[:, 0]]
    v2 = logits[ar, idx[:, 1]]
    e2 = np.exp(v2 - v1)
    w1 = (1.0 / (1.0 + e2)).astype(np.float32)
    w2 = (e2 / (1.0 + e2)).astype(np.float32)

    tok_lists, gate_lists = [], []
    for e in range(NE):
        s1 = np.nonzero(idx[:, 0] == e)[0]
        s2 = np.nonzero(idx[:, 1] == e)[0]
        tok_lists.append(np.concatenate([s1, s2]))
        gate_lists.append(np.concatenate([w1[s1], w2[s2]]))
    max_count = max(len(t) for t in tok_lists)
    mode = _dtype_mode()
    global LAST_CAP

    if mode == "fp16" and os.environ.get("MOE_SPLIT", "1") == "1":
        return _kernel_split(x, tok_lists, gate_lists, W_in, W_out, mode)

    cap_env = os.environ.get("MOE_CAP")
    cap = int(cap_env) if cap_env else -(-max_count // 4) * 4  # exact-ish
    if max_count > cap:
        cap = -(-max_count // 4) * 4
    cap = max(cap, 384)
    LAST_CAP = cap
    nc = _get_program(cap, mode)

    in_np = np.float16 if mode == "fp16" else np.float32
    mm2t = mode == "fp16" and os.environ.get("MOE_MM2T", "1") == "1"
    gate_np = np.float16 if mm2t else np.float32
    in_maps = []
    for e in range(NE):
        toks = tok_lists[e]
        xg = np.zeros((DH, cap), in_np)
        xg[:, : len(toks)] = x[toks].T.astype(in_np)
        if mm2t:
            g = np.zeros((cap,), gate_np)
            g[: len(toks)] = gate_lists[e].astype(gate_np)
            g = np.ascontiguousarray(np.broadcast_to(g, (P, cap)))
        else:
            g = np.zeros((-(-cap // P) * P,), gate_np)
            g[: len(toks)] = gate_lists[e].astype(gate_np)
        in_maps.append(
            {
                "xg": xg,
                "w_in_t": np.ascontiguousarray(W_in[e].T.astype(in_np)),
                "w_out": np.ascontiguousarray(W_out[e].astype(in_np)),
                "gate": g,
            }
        )

    trace = os.environ.get("MOE_TRACE", "0") == "1"
    res = run_bass_kernel_spmd(
        nc,
        in_maps,
        list(range(N_CORES)),
        trace=trace,
        trace_cores=list(range(N_CORES)) if trace else None,
    )
    LAST_EXEC_NS = res.exec_time_ns
    LAST_RESULTS = res

    out = np.zeros((T, DH), np.float32)
    for e in range(NE):
        toks = tok_lists[e]
        if len(toks):
            ye = res.results[e]["y"]
            if mm2t:
                ye = ye[:, : len(toks)].T
            else:
                ye = ye[: len(toks)]
            out[toks] += ye.astype(np.float32)
    return out



# revision 57
# speedup vs baseline: 1.0242x; 1.0242x over previous
"""MoE head (router top-2 + per-expert GELU FFN) on 8 TRN2 NeuronCores.

Strategy (sharding hint: expert parallel with top-k dispatch):
  - Host computes the (tiny) router in float64: logits = x @ Wr.T,
    top-2 experts per token, softmax-over-top2 gate weights.
  - Tokens are gathered per expert on the host ("all-to-all dispatch"),
    padded to a fixed capacity C, and each of the 8 cores runs ONE
    expert's FFN over its gathered tokens:
        y_tok = gate * (gelu(x_tok @ W_in[e].T) @ W_out[e])
  - Host scatter-adds the two expert contributions per token.

  Device kernel (SPMD, one program, per-core data):
    mm1: h^T[dhid, tok] = W_in^T-chunks.T @ x^T-chunks  (PE, accumulate K=512)
    gelu: ACT engine, PSUM -> SBUF (fp16)
    mm2: y^T[dh, tok] = W_out-chunks @ h^T-chunks       (PE, accumulate K=2048)
    gate: DVE elementwise multiply vs pre-broadcast G[p,tok], then DMA out.

  Matmuls run in float16 (host-cast inputs). Measured on HW (sustained,
  loop-slope): fp16/bf16 ~253 ns vs f32r ~294 ns per [128x128]@[128x512]
  matmul chain (fp16 hits 1 row/cycle @2.4GHz; f32r pays ~19% more per
  row), and fp16 inputs halve the HBM stream (10.4 -> 5.2 MB/core, fully
  hidden under compute vs ~12 us exposed for f32r). fp16 also halves the
  y drain (y is written fp16, upcast on host). mm2 is computed
  TRANSPOSED (y^T[dh, tok], moving dim = ragged token group) so both
  matmuls are MAC-optimal: rows = 64*cap each, no ceil(cap/128) subtile
  waste. The per-token gate is applied to y^T as an elementwise DVE
  multiply against a host-pre-broadcast G[p, tok] = gate[tok] tile
  (280KB fp16, free under the weight stream). End-to-end error vs the
  fp32 reference: ~5.5e-4 scale-relative absmax (tolerance 2e-2).

  Timing on HW (test.py loop-slope, quiet machine): ~62-67 us/kernel vs
  98.5-101.5 us for the f32r baseline (~1.5x). The estimator is noisy
  and upward-skewed (axon RPC jitter + shared-terminal contention:
  identical code sampled 61-88 us across windows), so test.py reports
  the median of 5 independent median-of-30 slope rounds. CoreSim
  (no_exec) budget at cap=1092: PE busy 58.45 us (MAC floor 58.24, zero
  mid-stream gaps), DMA-wait head ~3.3 us (2 serial dma issue+DGE
  chains; w_in lands before xg since Ldweights precedes the matmul),
  drain tail ~2.6 us (last-chunk PSUM accumulation split in half so
  half the DVE+DMA drain overlaps the final matmuls; groups sorted
  descending so the last drain is the smallest group). PSUM ps1=5/ps2=3
  and the staggered phase order (mm1 g0, mm1 g1, mm2 g0, ...) scanned
  optimal in sim. Rejected by measurement: finer-grained head DMA and
  off-SP DMA issue (each dma_start costs ~565 ns of sequencer issue
  time; more/finer descriptors push the whole stream later; <512B
  contiguous chunks transfer at half rate), on-device gate broadcast
  via K=1 matmul (schedule poison; host pre-broadcast is free),
  expert-pair token balancing (needs 2 weight sets/core -> 9.2MB
  stream re-exposes DMA), fp8 DoubleRow (2x PE rate but ~3-10% error
  >> 2e-2 tolerance), PE prewarm (sim models ramp from t=0).

  MOE_SPLIT=1 (the default) selects the 2-slot dhid-split variant:
  each core computes half the hidden dim (1024) of one big + one small
  expert, so slot caps are (max big, max small) = (1092, 1004) instead
  of every core paying the global max -- 5760 fewer PE rows (2.4us)
  with no weight duplication (two half-experts = one full expert's
  bytes); the host sums the two half-dhid partial y's per expert (gate
  is linear, applied in each half). Needs coarse multi-kc rearrange
  DMA descriptors: a first fine-grained version lost 7.9us to SP
  sequencer saturation (~48us of issue time at ~565ns per dma_start).
  Sim: 64.13 vs 64.29us for single-expert/core; HW: equal-or-better in
  every paired window (the +1.5MB/core stream stays hidden, incl. 8-core
  self-contention). rel err 5.31e-4. MOE_SPLIT=0 falls back to
  1-expert/core. Caveat: >2 loop-program executables in one process can
  crash the axon runtime -- bench configs in separate processes.
"""

import os

import numpy as np

P = 128
DH = 512
DHID = 2048
NE = 8
TOPK = 2
KC1 = DH // P  # k-chunks for mm1
KC2 = DHID // P  # k-chunks for mm2
N_CORES = 8

DEFAULT_CAP = 1152  # tokens per expert; E[count] = T*K/NE = 1024 for T=4096

_prog_cache: dict[tuple, object] = {}
LAST_EXEC_NS = None  # filled when MOE_TRACE=1
LAST_RESULTS = None
LAST_CAP = None


def _dtype_mode() -> str:
    return os.environ.get("MOE_DTYPE", "fp16")  # fp16 | f32r | bf16 | fp32


def _groups_of(c: int) -> list[int]:
    """Token groups: each in [256, 512] so the f32r matmul moving dim stays
    >= 256 (4x slower below). All groups except the last are multiples of
    128, so every group offset is 128-aligned; the last group absorbs any
    remainder (its final mm2 subtile is partial-M)."""
    rem = c % 128
    if rem:
        last = 256 + rem  # in (256, 384)
        body = c - last
    else:
        last = None
        body = c
    gs = []
    left = body
    while left > 640:
        gs.append(512)
        left -= 512
    if left > 512:  # 640+128k leftovers: split into two >=256 chunks
        hi = (left // 256) * 128
        gs.extend([left - hi, hi])
    elif left:
        gs.append(left)
    if last is not None:
        gs.append(last)
    if os.environ.get("MOE_G0FIRST", "0") == "1" and len(gs) > 2:
        # Smallest group first: the PE's first mm1 phase needs the fewest
        # input bytes, so it starts sooner after the DMA head.
        i = gs.index(min(gs[:-1]))
        gs.insert(0, gs.pop(i))
    if os.environ.get("MOE_GSORT", "1") == "1":
        # Largest first, smallest last: the final mm2 drain (DVE + y DMA
        # serial after the last matmul) scales with the last group's size.
        gs.sort(reverse=True)
    return gs


def _build_program(
    cap: int, mode: str, reps: int = 1, loop_n: int = 0, loop_scope: str = "all"
):
    """loop_n > 0 wraps the body in a runtime For_i loop (timing only).
    loop_scope="compute" hoists the DMA+round stage out of the loop."""
    import contextlib

    import concourse.mybir as mybir
    import concourse.tile as tile
    from concourse import bacc

    f32 = mybir.dt.float32
    mm_dt = {
        "f32r": mybir.dt.float32r,
        "bf16": mybir.dt.bfloat16,
        "fp16": mybir.dt.float16,
        "fp32": mybir.dt.float32,
    }[mode]

    nc = bacc.Bacc(None, target_bir_lowering=False, debug=False)
    # f32r inputs are declared as float32r (same 4-byte layout as fp32; numpy
    # side is np.float32). The PE truncates fp32 -> f32r internally, so raw
    # fp32 bytes DMA'd straight in give the same result as an explicit
    # rounding pass -- verified on HW. fp16 inputs are cast on the HOST
    # (np.float16) and DMA'd directly: half the HBM bytes, no on-device cast.
    if mode == "f32r":
        in_dt = mybir.dt.float32r
    elif mode == "fp16":
        in_dt = mybir.dt.float16
    else:
        in_dt = f32
    xg = nc.declare_dram_parameter("xg", [DH, cap], in_dt, isOutput=False)
    w_in_t = nc.declare_dram_parameter("w_in_t", [DH, DHID], in_dt, isOutput=False)
    w_out = nc.declare_dram_parameter("w_out", [DHID, DH], in_dt, isOutput=False)
    gcols = -(-cap // P)
    # mm2t: y is computed transposed (y^T[dh, tok]) so mm2's moving dim is
    # the ragged token group instead of DH -- no partial-subtile waste
    # (mm2 rows drop from ceil(cap/128)*128*16*4 to cap*64, MAC-optimal).
    # The per-token gate becomes a per-COLUMN scale, applied via a
    # PE-built broadcast tile G[p, tok] = gate[tok].
    mm2t = mode == "fp16" and os.environ.get("MOE_MM2T", "1") == "1"
    gate_dt = mybir.dt.float16 if mm2t else f32
    # mm2t: gate arrives pre-broadcast from the host as G[p, tok] = gate[tok]
    # (280KB fp16 -- negligible next to the 4MB weight stream) so no
    # on-device broadcast build is needed.
    gate_shape = [P, cap] if mm2t else [gcols * P]
    gate = nc.declare_dram_parameter("gate", gate_shape, gate_dt, isOutput=False)
    # fp16 y halves the output drain; the host upcasts + scatter-adds in f32.
    out_dt = mybir.dt.float16 if mode == "fp16" else f32
    y_shape = [DH, cap] if mm2t else [cap, DH]
    y = nc.declare_dram_parameter("y", y_shape, out_dt, isOutput=True)

    groups = _groups_of(cap)
    gelu = mybir.ActivationFunctionType.Gelu

    with tile.TileContext(nc) as tc:
        with (
            tc.tile_pool(name="persist", bufs=1) as persist,
            tc.tile_pool(name="stage", bufs=3) as stage,
            # hbuf=1 for very large caps (all-tokens-to-one-expert fallback)
            # so the SBUF budget still closes; slower but correct.
            tc.tile_pool(
                name="hbuf",
                bufs=int(os.environ.get("MOE_HBUF", "2" if cap <= 2304 else "1")),
            ) as hbuf,
            tc.tile_pool(name="ybuf", bufs=int(os.environ.get("MOE_YBUF", "4"))) as ybuf,
            tc.tile_pool(name="ps1", bufs=int(os.environ.get("MOE_PS1", "5")), space="PSUM") as ps1,
            tc.tile_pool(name="ps2", bufs=int(os.environ.get("MOE_PS2", "3")), space="PSUM") as ps2,
        ):
            state = {}

            def emit_loads():
                # ---- direct DMA into matmul-dtype tiles (no staging) ----
                if mode == "bf16":
                    # bf16 needs an on-device cast: stage fp32 then DVE copy.
                    xg_r = persist.tile([P, KC1, cap], mm_dt, tag="xg_r")
                    for kc in range(KC1):
                        stg = stage.tile([P, cap], f32, tag="stg_x")
                        nc.sync.dma_start(out=stg, in_=xg[kc * P : (kc + 1) * P, :])
                        nc.vector.tensor_copy(xg_r[:, kc, :], stg)
                    w_in_r = persist.tile([P, KC1, DHID], mm_dt, tag="w_in_r")
                    for kc in range(KC1):
                        stg = stage.tile([P, DHID], f32, tag="stg_wi")
                        nc.sync.dma_start(
                            out=stg, in_=w_in_t[kc * P : (kc + 1) * P, :]
                        )
                        nc.vector.tensor_copy(w_in_r[:, kc, :], stg)
                    w_out_r = persist.tile([P, KC2, DH], mm_dt, tag="w_out_r")
                    for kc in range(KC2):
                        stg = stage.tile([P, DH], f32, tag="stg_wo")
                        nc.sync.dma_start(
                            out=stg, in_=w_out[kc * P : (kc + 1) * P, :]
                        )
                        nc.vector.tensor_copy(w_out_r[:, kc, :], stg)
                elif os.environ.get("MOE_FINEDMA", "1") == "1":
                    # First-use-ordered fine-grained input stream: the PE's
                    # first work (mm1 group0, d0-3) needs only xg[:,g0] and
                    # w_in quarter q0 -- land those ~2MB first so the PE
                    # starts ~6us in instead of waiting on whole chunks.
                    xg_r = persist.tile([P, KC1, cap], in_dt, tag="xg_r")
                    w_in_r = persist.tile([P, KC1, DHID], in_dt, tag="w_in_r")
                    w_out_r = persist.tile([P, KC2, DH], in_dt, tag="w_out_r")
                    Q = DHID // 4
                    g_offs = []
                    o = 0
                    for gsz in groups:
                        g_offs.append((o, gsz))
                        o += gsz

                    def dma_xg(kc, gi, eng=None):
                        o, gsz = g_offs[gi]
                        (eng or nc.sync).dma_start(
                            out=xg_r[:, kc, o : o + gsz],
                            in_=xg[kc * P : (kc + 1) * P, o : o + gsz],
                        )

                    # Issue weight streams from otherwise-idle engine
                    # sequencers: each dma_start costs ~565-667ns of issue
                    # time on its engine, and SP serializes the whole input
                    # stream otherwise. ACT is idle until the first gelu
                    # (~4us in), DVE until the first mm2 (~20us in).
                    # MOE_ENG: 0 = all DMA issue on SP; 1 = w_in on ACT +
                    # w_out on Pool; 2 = only the head w_in chunks on ACT
                    # (ACT is idle before the first gelu; later issues would
                    # block the gelu stream), rest on SP.
                    eng_mode = os.environ.get("MOE_ENG", "0")
                    win_eng = nc.scalar if eng_mode == "1" else nc.sync
                    wout_eng = nc.gpsimd if eng_mode == "1" else nc.sync
                    head_eng = nc.scalar if eng_mode in ("1", "2") else nc.sync

                    def dma_win_cols(kc, c0, c1, eng=None):
                        (eng or win_eng).dma_start(
                            out=w_in_r[:, kc, c0:c1],
                            in_=w_in_t[kc * P : (kc + 1) * P, c0:c1],
                        )

                    def dma_win(kc, q):
                        dma_win_cols(kc, q * Q, (q + 1) * Q)

                    if os.environ.get("MOE_FINEHEAD", "0") == "1":
                        # Minimal first-matmul set: xg group0 + w_in d0 cols
                        # only (32KB/chunk fp16), so the PE starts earliest.
                        for kc in range(KC1):
                            dma_xg(kc, 0)
                            dma_win_cols(kc, 0, P)
                        for kc in range(KC1):
                            dma_win_cols(kc, P, Q)
                            dma_xg(kc, 1)
                    elif os.environ.get("MOE_HEADI", "1") == "1":
                        winfirst = os.environ.get("MOE_WINFIRST", "1") == "1"
                        # MOE_ENG=3: xg group0 issues on ACT (idle until the
                        # first gelu) concurrently with w_in q0 on SP, so the
                        # two first-matmul inputs don't serialize on one
                        # sequencer.
                        xg0_eng = nc.scalar if eng_mode == "3" else None
                        for kc in range(KC1):
                            # winfirst: the PE's Ldweights needs w_in before
                            # the matmul touches xg -- land it first.
                            if winfirst:
                                dma_win_cols(kc, 0, Q, eng=head_eng)
                                dma_xg(kc, 0, eng=xg0_eng)
                            else:
                                dma_xg(kc, 0, eng=xg0_eng)
                                dma_win_cols(kc, 0, Q, eng=head_eng)
                        for kc in range(KC1):
                            dma_win(kc, 1)
                            dma_xg(kc, 1)
                    else:
                        for kc in range(KC1):
                            dma_xg(kc, 0)
                        for kc in range(KC1):
                            dma_win(kc, 0)
                        for kc in range(KC1):
                            dma_win(kc, 1)
                        for kc in range(KC1):
                            dma_xg(kc, 1)
                    if os.environ.get("MOE_FINEHEAD", "0") == "1":
                        for kc in range(KC1):
                            dma_win(kc, 1)
                    for kc in range(KC1):
                        dma_win(kc, 2)
                    for kc in range(KC1):
                        dma_win(kc, 3)
                    for kc2 in range(KC2 // 2):
                        wout_eng.dma_start(
                            out=w_out_r[:, kc2 * 2 : (kc2 + 1) * 2, :],
                            in_=w_out.rearrange("(kc p) d -> p kc d", p=P)[
                                :, kc2 * 2 : (kc2 + 1) * 2, :
                            ],
                        )
                    for gi in range(2, len(groups)):
                        for kc in range(KC1):
                            dma_xg(kc, gi)
                else:
                    xg_r = persist.tile([P, KC1, cap], in_dt, tag="xg_r")
                    for kc in range(KC1):
                        nc.sync.dma_start(
                            out=xg_r[:, kc, :], in_=xg[kc * P : (kc + 1) * P, :]
                        )
                    w_in_r = persist.tile([P, KC1, DHID], in_dt, tag="w_in_r")
                    for kc in range(KC1):
                        for h2 in range(2):  # halves for earlier first-use
                            nc.sync.dma_start(
                                out=w_in_r[:, kc, h2 * (DHID // 2) : (h2 + 1) * (DHID // 2)],
                                in_=w_in_t[
                                    kc * P : (kc + 1) * P,
                                    h2 * (DHID // 2) : (h2 + 1) * (DHID // 2),
                                ],
                            )
                    w_out_r = persist.tile([P, KC2, DH], in_dt, tag="w_out_r")
                    for kc4 in range(KC2 // 4):
                        nc.sync.dma_start(
                            out=w_out_r[:, kc4 * 4 : (kc4 + 1) * 4, :],
                            in_=w_out.rearrange("(kc p) d -> p kc d", p=P)[
                                :, kc4 * 4 : (kc4 + 1) * 4, :
                            ],
                        )
                if mm2t:
                    gate_sb = persist.tile([P, cap], gate_dt, tag="gate_sb")
                    nc.sync.dma_start(out=gate_sb, in_=gate[:, :])
                else:
                    gate_sb = persist.tile([P, gcols], f32, tag="gate_sb")
                    nc.sync.dma_start(
                        out=gate_sb, in_=gate.rearrange("(g p) -> p g", p=P)
                    )
                state.update(
                    xg_r=xg_r, w_in_r=w_in_r, w_out_r=w_out_r, gate_sb=gate_sb
                )

            def emit_compute():
                # ---- mm1 -> gelu -> mm2 -> gate -> out, per token-group ----
                # Phase order is staggered (mm1 g0, mm1 g1, mm2 g0, mm1 g2,
                # mm2 g1, mm2 g2) so the first mm2 starts ~2 mm1-phases into
                # the kernel, giving the 4MB w_out DMA stream time to land
                # without stalling the PE. Needs 2 live h tiles (hbuf=2).
                xg_r, w_in_r = state["xg_r"], state["w_in_r"]
                w_out_r, gate_sb = state["w_out_r"], state["gate_sb"]
                offs = []
                off = 0
                for gsz in groups:
                    offs.append(off)
                    off += gsz
                h_tiles = {}

                g_bc = gate_sb if mm2t else None

                def mm1_phase(gi):
                    gsz, off = groups[gi], offs[gi]
                    h_r = hbuf.tile([P, KC2, gsz], mm_dt, tag="h_r")
                    h_tiles[gi] = h_r
                    for d in range(KC2):
                        ps = ps1.tile([P, gsz], f32, tag="p1")
                        for kc in range(KC1):
                            nc.tensor.matmul(
                                ps,
                                w_in_r[:, kc, d * P : (d + 1) * P],
                                xg_r[:, kc, off : off + gsz],
                                start=(kc == 0),
                                stop=(kc == KC1 - 1),
                            )
                        nc.scalar.activation(h_r[:, d, :], ps, gelu)

                def mm2_phase(gi, tail=False):
                    gsz, off = groups[gi], offs[gi]
                    h_r = h_tiles.pop(gi)
                    if mm2t:
                        # y^T[dh_c, tok] = sum_d W_out[d, dh_c]^T @ h^T[d, tok]
                        # moving dim = tokens (ragged-exact), then gate via
                        # elementwise multiply with the G broadcast tile.
                        tsplit = (
                            tail
                            and gsz > 128
                            and os.environ.get("MOE_TAILSPLIT", "1") == "1"
                        )
                        for c in range(KC1):
                            if tsplit and c == KC1 - 1:
                                # Split the very last chunk's accumulation so
                                # half its DVE+DMA drain overlaps the PE's
                                # final matmuls instead of trailing them.
                                h1 = gsz // 2 // 64 * 64
                                parts = [(0, h1), (h1, gsz - h1)]
                            else:
                                parts = [(0, gsz)]
                            for po, pl in parts:
                                pt = ps2.tile([P, pl], f32, tag="p2")
                                for d in range(KC2):
                                    nc.tensor.matmul(
                                        pt,
                                        w_out_r[:, d, c * P : (c + 1) * P],
                                        h_r[:, d, po : po + pl],
                                        start=(d == 0),
                                        stop=(d == KC2 - 1),
                                    )
                                y_sb = ybuf.tile([P, pl], out_dt, tag="y_sb")
                                nc.vector.scalar_tensor_tensor(
                                    y_sb,
                                    pt,
                                    1.0,
                                    g_bc[:, off + po : off + po + pl],
                                    mybir.AluOpType.mult,
                                    mybir.AluOpType.mult,
                                )
                                nc.sync.dma_start(
                                    out=y[
                                        c * P : (c + 1) * P,
                                        off + po : off + po + pl,
                                    ],
                                    in_=y_sb,
                                )
                        return
                    for s in range(-(-gsz // P)):
                        m = min(P, gsz - s * P)  # last subtile may be partial
                        pt = ps2.tile([P, DH], f32, tag="p2")
                        for d in range(KC2):
                            nc.tensor.matmul(
                                pt[:m, :],
                                h_r[:, d, s * P : s * P + m],
                                w_out_r[:, d, :],
                                start=(d == 0),
                                stop=(d == KC2 - 1),
                            )
                        tok0 = off + s * P
                        y_sb = ybuf.tile([P, DH], out_dt, tag="y_sb")
                        nc.vector.tensor_scalar_mul(
                            y_sb[:m, :], pt[:m, :], gate_sb[:m, tok0 // P : tok0 // P + 1]
                        )
                        nc.sync.dma_start(out=y[tok0 : tok0 + m, :], in_=y_sb[:m, :])

                n_g = len(groups)
                if n_g == 1 or os.environ.get("MOE_STAGGER", "1") != "1":
                    for gi in range(n_g):
                        mm1_phase(gi)
                        mm2_phase(gi, tail=(gi == n_g - 1))
                else:
                    mm1_phase(0)
                    mm1_phase(1)
                    for gi in range(2, n_g):
                        mm2_phase(gi - 2)
                        mm1_phase(gi)
                    mm2_phase(n_g - 2)
                    mm2_phase(n_g - 1, tail=True)

            if loop_n and loop_scope == "loads":
                # DMA-stream-only loop: measures the pure input stream time.
                with tc.For_i(0, loop_n, 1):
                    for _rep in range(reps):
                        emit_loads()
                emit_compute()
            elif loop_n and loop_scope == "compute":
                emit_loads()
                with tc.For_i(0, loop_n, 1):
                    for _rep in range(reps):
                        emit_compute()
            elif loop_n:
                with tc.For_i(0, loop_n, 1):
                    for _rep in range(reps):
                        emit_loads()
                        emit_compute()
            else:
                for _rep in range(reps):
                    emit_loads()
                    emit_compute()

    nc.compile()
    return nc


def _build_program_split(
    caps, mode: str = "fp16", loop_n: int = 0, loop_scope: str = "all"
):
    """2-slot dhid-split: each core runs HALF the hidden dim (1024) of two
    different experts -- slot0 one of the 4 biggest, slot1 one of the 4
    smallest -- so slot capacities are (max big, max small) instead of every
    core paying the global max. Same total weight bytes as 1-expert/core;
    only xg/gate/y are duplicated (~+1.5MB/core). Host sums the two half-dhid
    partial y's per expert (gate is linear, applied in each half)."""
    import concourse.mybir as mybir
    import concourse.tile as tile
    from concourse import bacc

    assert mode == "fp16"
    f32 = mybir.dt.float32
    fp16 = mybir.dt.float16
    DHH = DHID // 2
    KCH = DHH // P  # 8 k-chunks for mm2 per segment
    S = list(caps)

    nc = bacc.Bacc(None, target_bir_lowering=False, debug=False)
    xg = [
        nc.declare_dram_parameter(f"xg{s}", [DH, S[s]], fp16, isOutput=False)
        for s in range(2)
    ]
    wi = [
        nc.declare_dram_parameter(f"wi{s}", [DH, DHH], fp16, isOutput=False)
        for s in range(2)
    ]
    wo = [
        nc.declare_dram_parameter(f"wo{s}", [DHH, DH], fp16, isOutput=False)
        for s in range(2)
    ]
    gt = [
        nc.declare_dram_parameter(f"g{s}", [P, S[s]], fp16, isOutput=False)
        for s in range(2)
    ]
    y = [
        nc.declare_dram_parameter(f"y{s}", [DH, S[s]], fp16, isOutput=True)
        for s in range(2)
    ]
    groups = [_groups_of(S[s]) for s in range(2)]
    gelu = mybir.ActivationFunctionType.Gelu

    with tile.TileContext(nc) as tc:
        with (
            tc.tile_pool(name="persist", bufs=1) as persist,
            tc.tile_pool(
                name="hbuf", bufs=int(os.environ.get("MOE_HBUF", "2"))
            ) as hbuf,
            tc.tile_pool(
                name="ybuf", bufs=int(os.environ.get("MOE_YBUF", "4"))
            ) as ybuf,
            tc.tile_pool(
                name="ps1", bufs=int(os.environ.get("MOE_PS1", "5")), space="PSUM"
            ) as ps1,
            tc.tile_pool(
                name="ps2", bufs=int(os.environ.get("MOE_PS2", "3")), space="PSUM"
            ) as ps2,
        ):
            state = {}

            def emit_loads():
                QW = DHH // 4
                xg_r, wi_r, wo_r, g_r = [], [], [], []
                for s in range(2):
                    xg_r.append(
                        persist.tile(
                            [P, KC1, S[s]], fp16, tag=f"xg_r{s}", name=f"xg_r{s}"
                        )
                    )
                    wi_r.append(
                        persist.tile(
                            [P, KC1, DHH], fp16, tag=f"wi_r{s}", name=f"wi_r{s}"
                        )
                    )
                    wo_r.append(
                        persist.tile(
                            [P, KCH, DH], fp16, tag=f"wo_r{s}", name=f"wo_r{s}"
                        )
                    )
                    g_r.append(
                        persist.tile([P, S[s]], fp16, tag=f"g_r{s}", name=f"g_r{s}")
                    )
                offs = []
                for s in range(2):
                    o, oo = 0, []
                    for gsz in groups[s]:
                        oo.append(o)
                        o += gsz
                    offs.append(oo)

                # Coarse multi-kc descriptors (one dma_start per logical
                # block) everywhere except the fine seg0 head: each
                # dma_start costs ~565ns of SP issue time and the split
                # program has ~2x the blocks of the single-expert one.
                def dma_xg(s, gi):
                    o, gsz = offs[s][gi], groups[s][gi]
                    nc.sync.dma_start(
                        out=xg_r[s][:, :, o : o + gsz],
                        in_=xg[s].rearrange("(kc p) c -> p kc c", p=P)[
                            :, :, o : o + gsz
                        ],
                    )

                def dma_wi(s, q):
                    nc.sync.dma_start(
                        out=wi_r[s][:, :, q * QW : (q + 1) * QW],
                        in_=wi[s].rearrange("(kc p) c -> p kc c", p=P)[
                            :, :, q * QW : (q + 1) * QW
                        ],
                    )

                def dma_wo(s, half):
                    k2 = KCH // 2
                    nc.sync.dma_start(
                        out=wo_r[s][:, half * k2 : (half + 1) * k2, :],
                        in_=wo[s].rearrange("(kc p) d -> p kc d", p=P)[
                            :, half * k2 : (half + 1) * k2, :
                        ],
                    )

                # seg0 head: fine per-kc chunks, w_in before xg (winfirst).
                # MOE_HEADQ2=1 widens the head w_in chunks to q0+q1 (4
                # d-chunks of PE runway instead of 2) to cover the coarse
                # stream's arrival.
                hq = 2 * QW if os.environ.get("MOE_HEADQ2", "1") == "1" else QW
                for kc in range(KC1):
                    nc.sync.dma_start(
                        out=wi_r[0][:, kc, 0:hq], in_=wi[0][kc * P : (kc + 1) * P, 0:hq]
                    )
                    nc.sync.dma_start(
                        out=xg_r[0][:, kc, 0 : groups[0][0]],
                        in_=xg[0][kc * P : (kc + 1) * P, 0 : groups[0][0]],
                    )
                # All remaining w_in quarters straight after the head: the
                # 8-d mm1 burns a 2-d quarter every ~1.7us, while xg group1
                # isn't touched until ~9us in.
                if hq == QW:
                    dma_wi(0, 1)
                dma_wi(0, 2)
                dma_wi(0, 3)
                dma_xg(0, 1)
                dma_wo(0, 0)
                dma_wo(0, 1)
                for gi in range(2, len(groups[0])):
                    dma_xg(0, gi)
                nc.sync.dma_start(out=g_r[0], in_=gt[0][:, :])
                dma_wi(1, 0)
                dma_xg(1, 0)
                dma_wi(1, 1)
                dma_wi(1, 2)
                dma_wi(1, 3)
                dma_xg(1, 1)
                dma_wo(1, 0)
                dma_wo(1, 1)
                for gi in range(2, len(groups[1])):
                    dma_xg(1, gi)
                nc.sync.dma_start(out=g_r[1], in_=gt[1][:, :])
                state.update(xg_r=xg_r, wi_r=wi_r, wo_r=wo_r, g_r=g_r, offs=offs)

            def emit_compute():
                xg_r, wi_r = state["xg_r"], state["wi_r"]
                wo_r, g_r, offs = state["wo_r"], state["g_r"], state["offs"]
                h_tiles = {}

                def mm1_phase(s, gi):
                    gsz, off = groups[s][gi], offs[s][gi]
                    h_r = hbuf.tile([P, KCH, gsz], fp16, tag="h_r")
                    h_tiles[(s, gi)] = h_r
                    for d in range(KCH):
                        ps = ps1.tile([P, gsz], f32, tag="p1")
                        for kc in range(KC1):
                            nc.tensor.matmul(
                                ps,
                                wi_r[s][:, kc, d * P : (d + 1) * P],
                                xg_r[s][:, kc, off : off + gsz],
                                start=(kc == 0),
                                stop=(kc == KC1 - 1),
                            )
                        nc.scalar.activation(h_r[:, d, :], ps, gelu)

                def mm2_phase(s, gi, tail=False):
                    gsz, off = groups[s][gi], offs[s][gi]
                    h_r = h_tiles.pop((s, gi))
                    for c in range(KC1):
                        if tail and c == KC1 - 1 and gsz > 128:
                            h1 = gsz // 2 // 64 * 64
                            parts = [(0, h1), (h1, gsz - h1)]
                        else:
                            parts = [(0, gsz)]
                        for po, pl in parts:
                            pt = ps2.tile([P, pl], f32, tag="p2")
                            for d in range(KCH):
                                nc.tensor.matmul(
                                    pt,
                                    wo_r[s][:, d, c * P : (c + 1) * P],
                                    h_r[:, d, po : po + pl],
                                    start=(d == 0),
                                    stop=(d == KCH - 1),
                                )
                            y_sb = ybuf.tile([P, pl], fp16, tag="y_sb")
                            nc.vector.scalar_tensor_tensor(
                                y_sb,
                                pt,
                                1.0,
                                g_r[s][:, off + po : off + po + pl],
                                mybir.AluOpType.mult,
                                mybir.AluOpType.mult,
                            )
                            nc.sync.dma_start(
                                out=y[s][
                                    c * P : (c + 1) * P, off + po : off + po + pl
                                ],
                                in_=y_sb,
                            )

                phases = [(0, gi) for gi in range(len(groups[0]))] + [
                    (1, gi) for gi in range(len(groups[1]))
                ]
                n_p = len(phases)
                mm1_phase(*phases[0])
                mm1_phase(*phases[1])
                for i in range(2, n_p):
                    mm2_phase(*phases[i - 2])
                    mm1_phase(*phases[i])
                mm2_phase(*phases[n_p - 2])
                mm2_phase(*phases[n_p - 1], tail=True)

            if loop_n and loop_scope == "loads":
                with tc.For_i(0, loop_n, 1):
                    emit_loads()
                emit_compute()
            elif loop_n and loop_scope == "compute":
                emit_loads()
                with tc.For_i(0, loop_n, 1):
                    emit_compute()
            elif loop_n:
                with tc.For_i(0, loop_n, 1):
                    emit_loads()
                    emit_compute()
            else:
                emit_loads()
                emit_compute()

    nc.compile()
    return nc


def _get_program(cap, mode: str):
    key = (tuple(cap) if isinstance(cap, (tuple, list)) else cap, mode)
    if key not in _prog_cache:
        if isinstance(cap, (tuple, list)):
            _prog_cache[key] = _build_program_split(cap, mode)
        else:
            _prog_cache[key] = _build_program(cap, mode)
    return _prog_cache[key]


def _kernel_split(x, tok_lists, gate_lists, W_in, W_out, mode):
    """Dispatch for the 2-slot dhid-split program: core k runs half k%2 of
    big-expert ord[k//2] (slot0) and of small-expert ord[4+k//2] (slot1)."""
    global LAST_EXEC_NS, LAST_RESULTS, LAST_CAP
    from concourse.bass_utils import run_bass_kernel_spmd

    DHH = DHID // 2
    ordr = sorted(range(NE), key=lambda e: -len(tok_lists[e]))
    S1 = max(-(-len(tok_lists[ordr[i]]) // 4) * 4 for i in range(4))
    S2 = max(-(-len(tok_lists[ordr[i]]) // 4) * 4 for i in range(4, 8))
    S1, S2 = max(S1, 384), max(S2, 384)
    LAST_CAP = (S1, S2)
    nc = _get_program((S1, S2), mode)

    def seg_inputs(e, half, cap):
        toks = tok_lists[e]
        hs = slice(half * DHH, (half + 1) * DHH)
        xg = np.zeros((DH, cap), np.float16)
        xg[:, : len(toks)] = x[toks].T.astype(np.float16)
        g = np.zeros((cap,), np.float16)
        g[: len(toks)] = gate_lists[e].astype(np.float16)
        g = np.ascontiguousarray(np.broadcast_to(g, (P, cap)))
        return {
            "xg": xg,
            "wi": np.ascontiguousarray(W_in[e][hs].T.astype(np.float16)),
            "wo": np.ascontiguousarray(W_out[e][hs].astype(np.float16)),
            "g": g,
        }

    in_maps = []
    for k in range(N_CORES):
        m = {}
        for s, (e, cap) in enumerate(
            ((ordr[k // 2], S1), (ordr[4 + k // 2], S2))
        ):
            seg = seg_inputs(e, k % 2, cap)
            m.update({f"{name}{s}": v for name, v in seg.items()})
        in_maps.append(m)

    trace = os.environ.get("MOE_TRACE", "0") == "1"
    res = run_bass_kernel_spmd(
        nc,
        in_maps,
        list(range(N_CORES)),
        trace=trace,
        trace_cores=list(range(N_CORES)) if trace else None,
    )
    LAST_EXEC_NS = res.exec_time_ns
    LAST_RESULTS = res

    T = x.shape[0]
    out = np.zeros((T, DH), np.float32)
    for k in range(N_CORES):
        for s, e in enumerate((ordr[k // 2], ordr[4 + k // 2])):
            toks = tok_lists[e]
            if len(toks):
                out[toks] += res.results[k][f"y{s}"][:, : len(toks)].T.astype(
                    np.float32
                )
    return out


def kernel(x, Wr, W_in, W_out):
    global LAST_EXEC_NS, LAST_RESULTS
    from concourse.bass_utils import run_bass_kernel_spmd

    x = np.ascontiguousarray(np.asarray(x), dtype=np.float32)
    Wr = np.asarray(Wr, dtype=np.float32)
    W_in = np.asarray(W_in, dtype=np.float32)
    W_out = np.asarray(W_out, dtype=np.float32)
    T = x.shape[0]

    # ---- host router (fp64: strictly more accurate than the fp32 ref) ----
    logits = x.astype(np.float64) @ Wr.astype(np.float64).T  # (T, NE)
    part = np.argpartition(-logits, TOPK - 1, axis=1)[:, :TOPK]
    vals = np.take_along_axis(logits, part, axis=1)
    order = np.argsort(-vals, axis=1, kind="stable")
    idx = np.take_along_axis(part, order, axis=1)  # (T, 2) desc
    ar = np.arange(T)
    v1 = logits[ar, idx[:, 0]]
    v2 = logits[ar, idx[:, 1]]
    e2 = np.exp(v2 - v1)
    w1 = (1.0 / (1.0 + e2)).astype(np.float32)
    w2 = (e2 / (1.0 + e2)).astype(np.float32)

    tok_lists, gate_lists = [], []
    for e in range(NE):
        s1 = np.nonzero(idx[:, 0] == e)[0]
        s2 = np.nonzero(idx[:, 1] == e)[0]
        tok_lists.append(np.concatenate([s1, s2]))
        gate_lists.append(np.concatenate([w1[s1], w2[s2]]))
    max_count = max(len(t) for t in tok_lists)
    mode = _dtype_mode()
    global LAST_CAP

    if mode == "fp16" and os.environ.get("MOE_SPLIT", "1") == "1":
        return _kernel_split(x, tok_lists, gate_lists, W_in, W_out, mode)

    cap_env = os.environ.get("MOE_CAP")
    cap = int(cap_env) if cap_env else -(-max_count // 4) * 4  # exact-ish
    if max_count > cap:
        cap = -(-max_count // 4) * 4
    cap = max(cap, 384)
    LAST_CAP = cap
    nc = _get_program(cap, mode)

    in_np = np.float16 if mode == "fp16" else np.float32
    mm2t = mode == "fp16" and os.environ.get("MOE_MM2T", "1") == "1"
    gate_np = np.float16 if mm2t else np.float32
    in_maps = []
    for e in range(NE):
        toks = tok_lists[e]
        xg = np.zeros((DH, cap), in_np)
        xg[:, : len(toks)] = x[toks].T.astype(in_np)
        if mm2t:
            g = np.zeros((cap,), gate_np)
            g[: len(toks)] = gate_lists[e].astype(gate_np)
            g = np.ascontiguousarray(np.broadcast_to(g, (P, cap)))
        else:
            g = np.zeros((-(-cap // P) * P,), gate_np)
            g[: len(toks)] = gate_lists[e].astype(gate_np)
        in_maps.append(
            {
                "xg": xg,
                "w_in_t": np.ascontiguousarray(W_in[e].T.astype(in_np)),
                "w_out": np.ascontiguousarray(W_out[e].astype(in_np)),
                "gate": g,
            }
        )

    trace = os.environ.get("MOE_TRACE", "0") == "1"
    res = run_bass_kernel_spmd(
        nc,
        in_maps,
        list(range(N_CORES)),
        trace=trace,
        trace_cores=list(range(N_CORES)) if trace else None,
    )
    LAST_EXEC_NS = res.exec_time_ns
    LAST_RESULTS = res

    out = np.zeros((T, DH), np.float32)
    for e in range(NE):
        toks = tok_lists[e]
        if len(toks):
            ye = res.results[e]["y"]
            if mm2t:
                ye = ye[:, : len(toks)].T
            else:
                ye = ye[: len(toks)]
            out[toks] += ye.astype(np.float32)
    return out

